# revision 4
# baseline (speedup 1.0000x reference)
"""CompGCN classifier TRN2 kernel — 8-core edge/node-parallel, two launches.

Math (reference):
  rel_feat = rel_emb[lab];  h = [nf[src], nf[tgt], rel_feat]
  msg = gelu(h @ msg_W.T + msg_b)
  agg = segment_sum(msg, tgt, N)
  nf_u = GRUCell(agg, nf)
  c = [nf_u[src], nf_u[tgt], ef];  hc = gelu(c @ cls_W1.T + b1)
  out = hc @ cls_W2.T + b2

Strategy: fold the per-edge 384->128 matmuls into node-level table
precomputes (nfW1 = nf@W1m.T etc.), making the edge phases pure
gather+add+gelu.  Segment-sum via one-hot matmul into PSUM per
128-node block (edges pre-sorted by tgt on host, padded per block).
Launch A handles msg/agg/GRU/U-tables node-sharded; host concats the
U tables; launch B computes the edge classifier in original order.
"""
import sys
import types

sys.path.insert(0, '/opt/trn_rl_repo')
sys.path.insert(0, '/root/.axon_site')

import numpy as np
import concourse.bass as bass
import concourse.mybir as mybir
import concourse.tile as tile
import concourse.bass_utils as bu
from concourse.masks import make_identity

# zero-egress container: never upload profiling artifacts
bu.upload_artifacts = lambda tmpdir: 'local://' + tmpdir

# ---- problem constants (hardcoded per contract) ----
N, E, D = 50000, 500000, 128
NREL, NCLS = 64, 16
NC = 8
NPC = N // NC            # 6250 nodes per core
NB = 49                  # 128-node blocks per core (ceil(6250/128))
NODES_PAD = NB * 128     # 6272
TPB = 12                 # edge tiles per block (capacity 1536 edges/block)
BLK_CAP = TPB * 128
NPAD_G = 50176           # global padded node count (392*128)
E2 = E // NC             # 62500 classifier edges per core
T2 = 489                 # classifier tiles per core
E2P = T2 * 128           # 62592

dt = mybir.dt
F32 = dt.float32
I32 = dt.int32

TRACE = False
LAST_EXEC_NS = {}


def _split_multiwaits(nc, max_waits=1):
    """walrus in this image encodes at most 1 sem-wait per instruction; move
    excess waits onto preceding same-engine NoOps (same-engine program order
    preserves semantics)."""
    for b in nc.m.functions[0].blocks:
        old = list(b.instructions)
        new = []
        changed = False
        for inst in old:
            si = inst.sync_info
            if si is not None and len(si.on_wait) > max_waits:
                waits = list(si.on_wait)
                chunks = [waits[i:i + max_waits]
                          for i in range(0, len(waits), max_waits)]
                for k, ch in enumerate(chunks[:-1]):
                    new.append(mybir.InstNoOp(
                        name=f"{inst.name}_sw{k}", engine=inst.engine,
                        bass_nofuse=True,
                        sync_info=mybir.SyncInfo(on_wait=ch, on_update=[])))
                inst.sync_info = mybir.SyncInfo(
                    on_wait=chunks[-1], on_update=list(si.on_update))
                changed = True
            new.append(inst)
        if changed:
            b.instructions = new


def _make_nc():
    return bass.Bass("TRN2", target_bir_lowering=False, debug=False,
                     num_devices=NC)


def _build_A():
    nc = _make_nc()
    # inputs
    nfT = nc.dram_tensor("nfT", (D, NPAD_G), F32, kind="ExternalInput")
    nfTl = nc.dram_tensor("nfTl", (D, NODES_PAD), F32, kind="ExternalInput")
    W1mT = nc.dram_tensor("W1mT", (D, D), F32, kind="ExternalInput")
    W2mT = nc.dram_tensor("W2mT", (D, D), F32, kind="ExternalInput")
    Rrel = nc.dram_tensor("Rrel", (NREL, D), F32, kind="ExternalInput")
    wihT = nc.dram_tensor("wihT", (D, 3 * D), F32, kind="ExternalInput")
    whhT = nc.dram_tensor("whhT", (D, 3 * D), F32, kind="ExternalInput")
    gb = nc.dram_tensor("gb", (D, 4), F32, kind="ExternalInput")
    W1cT = nc.dram_tensor("W1cT", (D, D), F32, kind="ExternalInput")
    W2cT = nc.dram_tensor("W2cT", (D, D), F32, kind="ExternalInput")
    iota = nc.dram_tensor("iota", (D, D), F32, kind="ExternalInput")
    srcix = nc.dram_tensor("srcix", (D, NB * TPB), I32, kind="ExternalInput")
    tlocix = nc.dram_tensor("tlocix", (D, NB * TPB), I32, kind="ExternalInput")
    labix = nc.dram_tensor("labix", (D, NB * TPB), I32, kind="ExternalInput")
    trel = nc.dram_tensor("trel", (D, NB * TPB), F32, kind="ExternalInput")
    # outputs
    U1s = nc.dram_tensor("U1s", (NODES_PAD, D), F32, kind="ExternalOutput")
    U2s = nc.dram_tensor("U2s", (NODES_PAD, D), F32, kind="ExternalOutput")
    # scratch
    nfW1d = nc.dram_tensor("nfW1d", (NPAD_G, D), F32, kind="Internal")
    nfW2d = nc.dram_tensor("nfW2d", (NODES_PAD, D), F32, kind="Internal")

    with tile.TileContext(nc) as tc:
        with tc.tile_pool(name="const", bufs=1) as cp, \
             tc.tile_pool(name="pa", bufs=4) as pa, \
             tc.tile_pool(name="pb", bufs=3) as pb, \
             tc.tile_pool(name="gru", bufs=2) as pg, \
             tc.tile_pool(name="ps_agg", bufs=2, space="PSUM") as ps_agg, \
             tc.tile_pool(name="ps_a", bufs=2, space="PSUM") as ps_a, \
             tc.tile_pool(name="ps_g", bufs=2, space="PSUM") as ps_g, \
             tc.tile_pool(name="ps_u", bufs=2, space="PSUM") as ps_u:

            # constants in SBUF
            w1m = cp.tile([D, D], F32); nc.sync.dma_start(w1m[:], W1mT[:])
            w2m = cp.tile([D, D], F32); nc.sync.dma_start(w2m[:], W2mT[:])
            wih = cp.tile([D, 3 * D], F32); nc.sync.dma_start(wih[:], wihT[:])
            whh = cp.tile([D, 3 * D], F32); nc.sync.dma_start(whh[:], whhT[:])
            gbt = cp.tile([D, 4], F32); nc.sync.dma_start(gbt[:], gb[:])
            w1c = cp.tile([D, D], F32); nc.sync.dma_start(w1c[:], W1cT[:])
            w2c = cp.tile([D, D], F32); nc.sync.dma_start(w2c[:], W2cT[:])
            iot = cp.tile([D, D], F32); nc.sync.dma_start(iot[:], iota[:])
            six = cp.tile([D, NB * TPB], I32); nc.sync.dma_start(six[:], srcix[:])
            tlx = cp.tile([D, NB * TPB], I32); nc.sync.dma_start(tlx[:], tlocix[:])
            lbx = cp.tile([D, NB * TPB], I32); nc.sync.dma_start(lbx[:], labix[:])
            trl = cp.tile([D, NB * TPB], F32); nc.sync.dma_start(trl[:], trel[:])

            # ---- Phase A: node tables ----
            for i in range(NPAD_G // D):
                ch = pa.tile([D, D], F32)
                nc.sync.dma_start(ch[:], nfT[:, i * D:(i + 1) * D])
                p1 = ps_a.tile([D, D], F32, space="PSUM")
                nc.tensor.matmul(out=p1[:], lhsT=ch[:], rhs=w1m[:],
                                 start=True, stop=True)
                o1 = pa.tile([D, D], F32)
                nc.vector.tensor_copy(o1[:], p1[:])
                nc.sync.dma_start(nfW1d[i * D:(i + 1) * D, :], o1[:])
            for i in range(NB):
                ch = pa.tile([D, D], F32)
                nc.sync.dma_start(ch[:], nfTl[:, i * D:(i + 1) * D])
                p1 = ps_a.tile([D, D], F32, space="PSUM")
                nc.tensor.matmul(out=p1[:], lhsT=ch[:], rhs=w2m[:],
                                 start=True, stop=True)
                o1 = pa.tile([D, D], F32)
                nc.vector.tensor_copy(o1[:], p1[:])
                nc.sync.dma_start(nfW2d[i * D:(i + 1) * D, :], o1[:])

            # ---- Phase B: msg + segment-sum + GRU + U tables ----
            for b in range(NB):
                agg = ps_agg.tile([D, D], F32, space="PSUM")
                for t in range(TPB):
                    j = b * TPB + t
                    g1 = pb.tile([D, D], F32)
                    nc.gpsimd.indirect_dma_start(
                        out=g1[:], out_offset=None, in_=nfW1d[:],
                        in_offset=bass.IndirectOffsetOnAxis(
                            ap=six[:, j:j + 1], axis=0))
                    g2 = pb.tile([D, D], F32)
                    nc.gpsimd.indirect_dma_start(
                        out=g2[:], out_offset=None, in_=nfW2d[:],
                        in_offset=bass.IndirectOffsetOnAxis(
                            ap=tlx[:, j:j + 1], axis=0))
                    g3 = pb.tile([D, D], F32)
                    nc.gpsimd.indirect_dma_start(
                        out=g3[:], out_offset=None, in_=Rrel[:],
                        in_offset=bass.IndirectOffsetOnAxis(
                            ap=lbx[:, j:j + 1], axis=0))
                    s1 = pb.tile([D, D], F32)
                    nc.vector.tensor_add(out=s1[:], in0=g1[:], in1=g2[:])
                    s2 = pb.tile([D, D], F32)
                    nc.vector.tensor_add(out=s2[:], in0=s1[:], in1=g3[:])
                    msg = pb.tile([D, D], F32)
                    nc.scalar.activation(msg[:], s2[:],
                                         mybir.ActivationFunctionType.Gelu)
                    oh = pb.tile([D, D], F32)
                    nc.vector.tensor_scalar(
                        out=oh[:], in0=iot[:], scalar1=trl[:, j:j + 1],
                        scalar2=None, op0=mybir.AluOpType.is_equal)
                    nc.tensor.matmul(out=agg[:], lhsT=msg[:], rhs=oh[:],
                                     start=(t == 0), stop=(t == TPB - 1))
                # GRU for this block (128 nodes)
                aggs = pg.tile([D, D], F32)
                nc.scalar.copy(aggs[:], agg[:])
                nfb = pg.tile([D, D], F32)
                nc.sync.dma_start(nfb[:], nfTl[:, b * D:(b + 1) * D])
                g4 = ps_g.tile([D, 4 * D], F32, space="PSUM")
                nc.tensor.matmul(out=g4[:, 0:D], lhsT=wih[:, 0:D],
                                 rhs=aggs[:], start=True, stop=False)
                nc.tensor.matmul(out=g4[:, 0:D], lhsT=whh[:, 0:D],
                                 rhs=nfb[:], start=False, stop=True)
                nc.tensor.matmul(out=g4[:, D:2 * D], lhsT=wih[:, D:2 * D],
                                 rhs=aggs[:], start=True, stop=False)
                nc.tensor.matmul(out=g4[:, D:2 * D], lhsT=whh[:, D:2 * D],
                                 rhs=nfb[:], start=False, stop=True)
                nc.tensor.matmul(out=g4[:, 2 * D:3 * D], lhsT=wih[:, 2 * D:3 * D],
                                 rhs=aggs[:], start=True, stop=True)
                nc.tensor.matmul(out=g4[:, 3 * D:4 * D], lhsT=whh[:, 2 * D:3 * D],
                                 rhs=nfb[:], start=True, stop=True)
                r = pg.tile([D, D], F32)
                nc.scalar.activation(r[:], g4[:, 0:D],
                                     mybir.ActivationFunctionType.Sigmoid,
                                     bias=gbt[:, 0:1])
                z = pg.tile([D, D], F32)
                nc.scalar.activation(z[:], g4[:, D:2 * D],
                                     mybir.ActivationFunctionType.Sigmoid,
                                     bias=gbt[:, 1:2])
                hnb = pg.tile([D, D], F32)
                nc.scalar.add(hnb[:], g4[:, 3 * D:4 * D], gbt[:, 3:4])
                t1 = pg.tile([D, D], F32)
                nc.vector.tensor_mul(out=t1[:], in0=r[:], in1=hnb[:])
                t2 = pg.tile([D, D], F32)
                nc.vector.tensor_add(out=t2[:], in0=t1[:], in1=g4[:, 2 * D:3 * D])
                n_ = pg.tile([D, D], F32)
                nc.scalar.activation(n_[:], t2[:],
                                     mybir.ActivationFunctionType.Tanh,
                                     bias=gbt[:, 2:3])
                d1 = pg.tile([D, D], F32)
                nc.vector.tensor_tensor(out=d1[:], in0=nfb[:], in1=n_[:],
                                        op=mybir.AluOpType.subtract)
                d2 = pg.tile([D, D], F32)
                nc.vector.tensor_mul(out=d2[:], in0=z[:], in1=d1[:])
                nfu = pg.tile([D, D], F32)
                nc.vector.tensor_add(out=nfu[:], in0=n_[:], in1=d2[:])
                # U tables for this block
                pu = ps_u.tile([D, 2 * D], F32, space="PSUM")
                nc.tensor.matmul(out=pu[:, 0:D], lhsT=nfu[:], rhs=w1c[:],
                                 start=True, stop=True)
                nc.tensor.matmul(out=pu[:, D:2 * D], lhsT=nfu[:], rhs=w2c[:],
                                 start=True, stop=True)
                u1 = pg.tile([D, D], F32)
                nc.vector.tensor_copy(u1[:], pu[:, 0:D])
                nc.sync.dma_start(U1s[b * D:(b + 1) * D, :], u1[:])
                u2 = pg.tile([D, D], F32)
                nc.vector.tensor_copy(u2[:], pu[:, D:2 * D])
                nc.sync.dma_start(U2s[b * D:(b + 1) * D, :], u2[:])

    _split_multiwaits(nc)
    return nc


def _build_B():
    nc = _make_nc()
    U1 = nc.dram_tensor("U1", (N, D), F32, kind="ExternalInput")
    U2 = nc.dram_tensor("U2", (N, D), F32, kind="ExternalInput")
    efT = nc.dram_tensor("efT", (D, E2P), F32, kind="ExternalInput")
    s2T = nc.dram_tensor("s2T", (D, T2), I32, kind="ExternalInput")
    t2T = nc.dram_tensor("t2T", (D, T2), I32, kind="ExternalInput")
    W3cT = nc.dram_tensor("W3cT", (D, D), F32, kind="ExternalInput")
    clsW2T = nc.dram_tensor("clsW2T", (D, NCLS), F32, kind="ExternalInput")
    b1 = nc.dram_tensor("b1", (D, 1), F32, kind="ExternalInput")
    b2 = nc.dram_tensor("b2", (NCLS, 1), F32, kind="ExternalInput")
    outT = nc.dram_tensor("outT", (NCLS, E2P), F32, kind="ExternalOutput")

    with tile.TileContext(nc) as tc:
        with tc.tile_pool(name="const", bufs=1) as cp, \
             tc.tile_pool(name="p", bufs=3) as pp, \
             tc.tile_pool(name="ps1", bufs=2, space="PSUM") as ps1, \
             tc.tile_pool(name="ps2", bufs=2, space="PSUM") as ps2, \
             tc.tile_pool(name="ps3", bufs=2, space="PSUM") as ps3:
            w3 = cp.tile([D, D], F32); nc.sync.dma_start(w3[:], W3cT[:])
            w2 = cp.tile([D, NCLS], F32); nc.sync.dma_start(w2[:], clsW2T[:])
            b1t = cp.tile([D, 1], F32); nc.sync.dma_start(b1t[:], b1[:])
            b2t = cp.tile([NCLS, 1], F32); nc.sync.dma_start(b2t[:], b2[:])
            sx = cp.tile([D, T2], I32); nc.sync.dma_start(sx[:], s2T[:])
            tx = cp.tile([D, T2], I32); nc.sync.dma_start(tx[:], t2T[:])
            ident = cp.tile([D, D], F32)
            make_identity(nc, ident[:])

            for t in range(T2):
                ef = pp.tile([D, D], F32)
                nc.sync.dma_start(ef[:], efT[:, t * D:(t + 1) * D])
                g1 = pp.tile([D, D], F32)
                nc.gpsimd.indirect_dma_start(
                    out=g1[:], out_offset=None, in_=U1[:],
                    in_offset=bass.IndirectOffsetOnAxis(
                        ap=sx[:, t:t + 1], axis=0))
                g2 = pp.tile([D, D], F32)
                nc.gpsimd.indirect_dma_start(
                    out=g2[:], out_offset=None, in_=U2[:],
                    in_offset=bass.IndirectOffsetOnAxis(
                        ap=tx[:, t:t + 1], axis=0))
                pre = ps1.tile([D, D], F32, space="PSUM")
                nc.tensor.matmul(out=pre[:], lhsT=ef[:], rhs=w3[:],
                                 start=True, stop=True)
                s1 = pp.tile([D, D], F32)
                nc.vector.tensor_add(out=s1[:], in0=g1[:], in1=g2[:])
                s2 = pp.tile([D, D], F32)
                nc.vector.tensor_add(out=s2[:], in0=s1[:], in1=pre[:])
                hc = pp.tile([D, D], F32)
                nc.scalar.activation(hc[:], s2[:],
                                     mybir.ActivationFunctionType.Gelu,
                                     bias=b1t[:, 0:1])
                hT = ps2.tile([D, D], F32, space="PSUM")
                nc.tensor.transpose(out=hT[:], in_=hc[:], identity=ident[:])
                hTs = pp.tile([D, D], F32)
                nc.vector.tensor_copy(hTs[:], hT[:])
                po = ps3.tile([NCLS, D], F32, space="PSUM")
                nc.tensor.matmul(out=po[:], lhsT=w2[:], rhs=hTs[:],
                                 start=True, stop=True)
                ot = pp.tile([NCLS, D], F32)
                nc.scalar.add(ot[:], po[:], b2t[:, 0:1])
                nc.sync.dma_start(outT[:, t * D:(t + 1) * D], ot[:])

    _split_multiwaits(nc)
    return nc


_CACHE = {}


def _get(name, builder):
    if name not in _CACHE:
        _CACHE[name] = builder()
    return _CACHE[name]


def _run(nc, in_maps, tag):
    kw = {}
    if TRACE:
        import tempfile
        kw = dict(trace=True, tmpdir=tempfile.mkdtemp(prefix=f"gcn_{tag}_"))
    res = bu.run_bass_kernel_spmd(nc, in_maps, core_ids=list(range(NC)), **kw)
    if TRACE:
        LAST_EXEC_NS[tag] = res.exec_time_ns
        LAST_EXEC_NS[tag + "_dir"] = kw["tmpdir"]
    return res.results


def kernel(node_features, edge_features, edge_index, labels_for_rel_emb,
           rel_emb, msg_W, msg_b, gru_w_ih, gru_w_hh, gru_b_ih, gru_b_hh,
           cls_W1, cls_b1, cls_W2, cls_b2):
    nf = np.asarray(node_features, np.float32)
    ef = np.asarray(edge_features, np.float32)
    src = np.asarray(edge_index[0], np.int64).astype(np.int32)
    tgt = np.asarray(edge_index[1], np.int64).astype(np.int32)
    lab = np.asarray(labels_for_rel_emb, np.int64).astype(np.int32)
    msg_W = np.asarray(msg_W, np.float32)
    cls_W1 = np.asarray(cls_W1, np.float32)

    # ---- host preprocessing ----
    order = np.argsort(tgt, kind="stable")
    tgt_s = tgt[order]; src_s = src[order]; lab_s = lab[order]
    core = tgt_s // NPC
    blk = (tgt_s - core * NPC) // 128
    key = core * NB + blk
    gstart = np.searchsorted(key, np.arange(NC * NB + 1))
    counts = np.diff(gstart)
    assert counts.max() <= BLK_CAP, f"block overflow: {counts.max()}"
    slot = np.arange(len(key)) - gstart[key]
    gslot = key * BLK_CAP + slot
    TOT = NC * NB * BLK_CAP
    SRCP = np.zeros(TOT, np.int32); SRCP[gslot] = src_s
    TLOCP = np.zeros(TOT, np.int32); TLOCP[gslot] = tgt_s - core * NPC
    LABP = np.zeros(TOT, np.int32); LABP[gslot] = lab_s
    TRELP = np.full(TOT, -1.0, np.float32)
    TRELP[gslot] = (tgt_s - (core * NPC + blk * 128)).astype(np.float32)

    def perm_layout(a):
        # [NB*BLK_CAP] per core -> [128, NB*TPB] with col j = b*TPB+t
        return np.ascontiguousarray(
            a.reshape(NB, TPB, 128).transpose(2, 0, 1).reshape(128, NB * TPB))

    nfT = np.zeros((D, NPAD_G), np.float32)
    nfT[:, :N] = nf.T
    W1mT = np.ascontiguousarray(msg_W[:, 0:D].T)
    W2mT = np.ascontiguousarray(msg_W[:, D:2 * D].T)
    Rrel = (np.asarray(rel_emb, np.float32) @ msg_W[:, 2 * D:3 * D].T
            + np.asarray(msg_b, np.float32)).astype(np.float32)
    wihT = np.ascontiguousarray(np.asarray(gru_w_ih, np.float32).T)
    whhT = np.ascontiguousarray(np.asarray(gru_w_hh, np.float32).T)
    bih = np.asarray(gru_b_ih, np.float32); bhh = np.asarray(gru_b_hh, np.float32)
    gb = np.stack([bih[0:D] + bhh[0:D], bih[D:2 * D] + bhh[D:2 * D],
                   bih[2 * D:3 * D], bhh[2 * D:3 * D]], axis=1).astype(np.float32)
    W1cT = np.ascontiguousarray(cls_W1[:, 0:D].T)
    W2cT = np.ascontiguousarray(cls_W1[:, D:2 * D].T)
    W3cT = np.ascontiguousarray(cls_W1[:, 2 * D:3 * D].T)
    iota = np.broadcast_to(np.arange(D, dtype=np.float32), (D, D)).copy()

    in_maps_A = []
    for c in range(NC):
        lo = c * NB * BLK_CAP
        hi = lo + NB * BLK_CAP
        nfTl = np.zeros((D, NODES_PAD), np.float32)
        w = min(NODES_PAD, NPAD_G - c * NPC)
        nfTl[:, :w] = nfT[:, c * NPC:c * NPC + w]
        in_maps_A.append({
            "nfT": nfT, "nfTl": nfTl, "W1mT": W1mT, "W2mT": W2mT,
            "Rrel": Rrel, "wihT": wihT, "whhT": whhT, "gb": gb,
            "W1cT": W1cT, "W2cT": W2cT, "iota": iota,
            "srcix": perm_layout(SRCP[lo:hi]),
            "tlocix": perm_layout(TLOCP[lo:hi]),
            "labix": perm_layout(LABP[lo:hi]),
            "trel": perm_layout(TRELP[lo:hi]),
        })

    ncA = _get("A", _build_A)
    resA = _run(ncA, in_maps_A, "A")

    U1 = np.concatenate([resA[c]["U1s"][:NPC] for c in range(NC)], axis=0)
    U2 = np.concatenate([resA[c]["U2s"][:NPC] for c in range(NC)], axis=0)
    U1 = np.ascontiguousarray(U1)
    U2 = np.ascontiguousarray(U2)

    clsW2T = np.ascontiguousarray(np.asarray(cls_W2, np.float32).T)
    b1 = np.asarray(cls_b1, np.float32).reshape(D, 1)
    b2 = np.asarray(cls_b2, np.float32).reshape(NCLS, 1)

    in_maps_B = []
    for c in range(NC):
        sl = slice(c * E2, (c + 1) * E2)
        efTc = np.zeros((D, E2P), np.float32)
        efTc[:, :E2] = ef[sl].T
        sp = np.zeros(E2P, np.int32); sp[:E2] = src[sl]
        tp = np.zeros(E2P, np.int32); tp[:E2] = tgt[sl]
        in_maps_B.append({
            "U1": U1, "U2": U2, "efT": efTc,
            "s2T": np.ascontiguousarray(sp.reshape(T2, 128).T),
            "t2T": np.ascontiguousarray(tp.reshape(T2, 128).T),
            "W3cT": W3cT, "clsW2T": clsW2T, "b1": b1, "b2": b2,
        })

    ncB = _get("B", _build_B)
    resB = _run(ncB, in_maps_B, "B")

    out = np.concatenate(
        [resB[c]["outT"][:, :E2].T for c in range(NC)], axis=0)
    return np.ascontiguousarray(out.astype(np.float32))
